# revision 16
# baseline (speedup 1.0000x reference)
"""ATSS assignment kernel for Trainium2 (Bass/Tile), 8-core data-parallel.

One image per NeuronCore. Per-core pipeline:
  1. PE matmul computes approximate -dist^2 [gt, anchor] via the expanded
     bilinear form (fp32; ~0.25 abs error from catastrophic cancellation --
     fine for prefiltering only).
  2. DVE max8/max_index tournament: top-8 per 1024-anchor chunk, merged to
     [64 gt, 784] candidates, then iterative max8+match_replace to top-40
     per gt (margin over the exact top-27).
  3. Exact f32 dist^2 recomputed on the 40 candidates (same op order as the
     reference), exact top-27 selected via a match_replace marker trick.
  4. IoU/center-inside/mean/std on the candidate grid; positives scattered
     into the dense outputs with indirect DMAs (compute_op=max resolves
     anchors claimed by multiple gts; gather-back + equality finds winners).
  5. Bulk outputs are broadcast-filled by DMA (token_labels dominates:
     ~102 MB/core) -- that fill is the roofline term.
"""
import numpy as np

import concourse.bacc as bacc
import concourse.bass as bass
import concourse.mybir as mybir
import concourse.tile as tile
from concourse.bass import IndirectOffsetOnAxis
from concourse.tile import add_dep_helper
from concourse import bass_utils

dt = mybir.dt
F32 = dt.float32
I32 = dt.int32
U32 = dt.uint32
Alu = mybir.AluOpType
Act = mybir.ActivationFunctionType

N = 100000          # anchors per image
G = 64              # gts per image
T = 256             # token dim
B = 8               # images == cores
PW = 784            # anchors per partition (128*784 = 100352 padded)
NP = 128 * PW       # 100352
HALF = NP // 2      # 50176
CPAIR = 49          # chunk pairs of 1024 anchors per half
K2 = 40             # stage-2 candidates kept per gt
PK = K2 // 2        # packed fixup columns ([128, PK])
NPASS = 2           # scatter fixup passes (max duplicate multiplicity)
TOPK = 27
NEG_INF = -100000000.0   # reference INF
SENT = -3.0e38           # match_replace marker
DEAD = -1.0e30           # never-matching filler for match_replace
BIG_OFF = 4.0e6          # offset sentinel -> bounds-check skips it (x256 fits int32)


def _emit(tc, ctx, A, GTS, TOK, OV, OI, OM, OT, FD, LTD, CSD):
    nc = tc.nc
    cp = ctx.enter_context(tc.tile_pool(name="cp", bufs=1))
    rp = ctx.enter_context(tc.tile_pool(name="rp", bufs=4))
    pp = ctx.enter_context(tc.tile_pool(name="pp", bufs=4, space="PSUM"))

    # ---------------- output fills (start immediately) ----------------
    fill_insts = {}
    valc = cp.tile([1, 512], F32)
    nc.vector.memset(valc[:], NEG_INF)
    fi = nc.sync.dma_start(OV[:], valc[0:1, 0:160].unsqueeze(1).to_broadcast([1, 625, 160]))
    fill_insts['v'] = [fi]
    idxc = cp.tile([1, 512], I32)
    nc.vector.memset(idxc[:], 0)
    fi = nc.sync.dma_start(OI[:], idxc[0:1, 0:160].unsqueeze(1).to_broadcast([1, 625, 160]))
    fill_insts['i'] = [fi]

    # gts tile (also used for matched_gts fill + scatter source)
    GT = cp.tile([G, 4], F32)
    nc.sync.dma_start(GT[:], GTS[:])
    fill_insts['m'] = []
    for j in range(2):
        fi = nc.sync.dma_start(OM[j * 50000:(j + 1) * 50000, :],
                               GT[0:1, :].unsqueeze(1).to_broadcast([1, 50000, 4]))
        fill_insts['m'].append(fi)
    unit = cp.tile([1, T], F32)
    nc.vector.memset(unit[:], 0.0)
    nc.vector.memset(unit[:, T - 1:T], 1.0)
    fill_insts['t'] = []
    TFILL_CH = 8
    rows = N // TFILL_CH  # 12500
    for j in range(TFILL_CH):
        fi = nc.sync.dma_start(OT[j * rows:(j + 1) * rows, :],
                               unit[:].unsqueeze(1).to_broadcast([1, rows, T]))
        fill_insts['t'].append(fi)

    TOKT = cp.tile([G, T], F32)
    nc.sync.dma_start(TOKT[:], TOK[:])

    # ---------------- anchor load + features ----------------
    A4 = cp.tile([128, PW * 4], F32)
    nc.sync.dma_start(A4[0:127, :], A[0:127 * PW, :].rearrange("(p a) f -> p a f", a=PW))
    nc.sync.dma_start(A4[127:128, 0:(N - 127 * PW) * 4], A[127 * PW:N, :])
    # pad coords -> far away so padded anchors never win (DMA: compute ops
    # can't start at partition 127)
    padc = cp.tile([1, PW * 4 - (N - 127 * PW) * 4], F32)
    nc.vector.memset(padc[:], -4.0e6)
    nc.sync.dma_start(A4[127:128, (N - 127 * PW) * 4:], padc[:])

    v = A4[:].rearrange("p (a f) -> p a f", f=4)
    fcx = cp.tile([128, PW], F32)
    fcy = cp.tile([128, PW], F32)
    fr2 = cp.tile([128, PW], F32)
    ftmp = cp.tile([128, PW], F32)
    nc.vector.tensor_tensor(out=fcx[:], in0=v[:, :, 2], in1=v[:, :, 0], op=Alu.add)
    nc.vector.tensor_scalar_mul(fcx[:], fcx[:], 0.5)
    nc.vector.tensor_tensor(out=fcy[:], in0=v[:, :, 3], in1=v[:, :, 1], op=Alu.add)
    nc.vector.tensor_scalar_mul(fcy[:], fcy[:], 0.5)
    nc.vector.tensor_tensor(out=fr2[:], in0=fcx[:], in1=fcx[:], op=Alu.mult)
    nc.vector.tensor_tensor(out=ftmp[:], in0=fcy[:], in1=fcy[:], op=Alu.mult)
    nc.vector.tensor_tensor(out=fr2[:], in0=fr2[:], in1=ftmp[:], op=Alu.add)
    onest = cp.tile([128, PW], F32)
    nc.vector.memset(onest[:], 1.0)
    nc.sync.dma_start(FD[0:1, :], fcx[:])
    nc.sync.dma_start(FD[1:2, :], fcy[:])
    nc.sync.dma_start(FD[2:3, :], fr2[:])
    nc.sync.dma_start(FD[3:4, :], onest[:])

    # ---------------- gt features / lhsT (negated so matmul yields -d2) ----
    gcx = cp.tile([G, 1], F32)
    gcy = cp.tile([G, 1], F32)
    gar = cp.tile([G, 1], F32)
    gt1 = cp.tile([G, 1], F32)
    gt2 = cp.tile([G, 1], F32)
    nc.vector.tensor_tensor(out=gcx[:], in0=GT[:, 2:3], in1=GT[:, 0:1], op=Alu.add)
    nc.vector.tensor_scalar_mul(gcx[:], gcx[:], 0.5)
    nc.vector.tensor_tensor(out=gcy[:], in0=GT[:, 3:4], in1=GT[:, 1:2], op=Alu.add)
    nc.vector.tensor_scalar_mul(gcy[:], gcy[:], 0.5)
    nc.vector.tensor_tensor(out=gt1[:], in0=GT[:, 2:3], in1=GT[:, 0:1], op=Alu.subtract)
    nc.vector.tensor_tensor(out=gt2[:], in0=GT[:, 3:4], in1=GT[:, 1:2], op=Alu.subtract)
    nc.vector.tensor_tensor(out=gar[:], in0=gt1[:], in1=gt2[:], op=Alu.mult)
    # lhsT rows: 2*gcx, 2*gcy, -1, -(gcx^2+gcy^2)
    l0 = cp.tile([G, 1], F32)
    l1 = cp.tile([G, 1], F32)
    l3 = cp.tile([G, 1], F32)
    nc.vector.tensor_scalar_mul(l0[:], gcx[:], 2.0)
    nc.vector.tensor_scalar_mul(l1[:], gcy[:], 2.0)
    nc.vector.tensor_tensor(out=gt1[:], in0=gcx[:], in1=gcx[:], op=Alu.mult)
    nc.vector.tensor_tensor(out=gt2[:], in0=gcy[:], in1=gcy[:], op=Alu.mult)
    nc.vector.tensor_tensor(out=l3[:], in0=gt1[:], in1=gt2[:], op=Alu.add)
    nc.vector.tensor_scalar_mul(l3[:], l3[:], -1.0)
    negc = cp.tile([G, 1], F32)
    nc.vector.memset(negc[:], -1.0)
    nc.sync.dma_start(LTD[0:1, :], l0[:])
    nc.sync.dma_start(LTD[1:2, :], l1[:])
    nc.sync.dma_start(LTD[2:3, :], negc[:])
    nc.sync.dma_start(LTD[3:4, :], l3[:])
    LT = cp.tile([4, G], F32)
    nc.sync.dma_start(LT[:], LTD[:])

    # ---------------- stage 1: matmul -d2 + per-chunk top8 ----------------
    c_val = cp.tile([128, CPAIR * 8], F32)
    c_pos = cp.tile([128, CPAIR * 8], U32)
    FDH = FD[:].rearrange("k (h w) -> k h w", h=2)
    for i in range(CPAIR):
        rh = rp.tile([4, 2048], F32, tag="rh")
        nc.sync.dma_start(rh[:], FDH[:, :, i * 1024:(i + 1) * 1024])
        ps = pp.tile([128, 1024], F32, tag="ps")
        nc.tensor.matmul(ps[0:64, 0:512], LT[:], rh[:, 0:512], start=True, stop=True)
        nc.tensor.matmul(ps[0:64, 512:1024], LT[:], rh[:, 512:1024], start=True, stop=True)
        nc.tensor.matmul(ps[64:128, 0:512], LT[:], rh[:, 1024:1536], start=True, stop=True)
        nc.tensor.matmul(ps[64:128, 512:1024], LT[:], rh[:, 1536:2048], start=True, stop=True)
        nc.vector.max(out=c_val[:, i * 8:(i + 1) * 8], in_=ps[:])
        nc.vector.max_index(out=c_pos[:, i * 8:(i + 1) * 8],
                            in_max=c_val[:, i * 8:(i + 1) * 8], in_values=ps[:])

    # ---------------- candidate global ids ----------------
    c_gidx = cp.tile([128, CPAIR * 8], F32)
    nc.vector.tensor_copy(out=c_gidx[:], in_=c_pos[:])
    ib = cp.tile([128, CPAIR * 8], I32)
    nc.gpsimd.iota(ib[:], pattern=[[1024, CPAIR], [0, 8]], base=0, channel_multiplier=0)
    ibf = cp.tile([128, CPAIR * 8], F32)
    nc.vector.tensor_copy(out=ibf[:], in_=ib[:])
    nc.vector.tensor_scalar_add(ibf[64:128, :], ibf[64:128, :], float(HALF))
    nc.vector.tensor_tensor(out=c_gidx[:], in0=c_gidx[:], in1=ibf[:], op=Alu.add)

    # ---------------- merge halves -> [64, 784] ----------------
    S1 = CPAIR * 8  # 392
    cm_val = cp.tile([G, 2 * S1], F32)
    cm_idx = cp.tile([G, 2 * S1], F32)
    nc.sync.dma_start(cm_val[:, 0:S1], c_val[0:64, :])
    nc.sync.dma_start(cm_val[:, S1:2 * S1], c_val[64:128, :])
    nc.sync.dma_start(cm_idx[:, 0:S1], c_gidx[0:64, :])
    nc.sync.dma_start(cm_idx[:, S1:2 * S1], c_gidx[64:128, :])

    # ---------------- stage 2: top-40 per gt ----------------
    s2v = cp.tile([G, K2], F32)
    s2p = cp.tile([G, K2], U32)
    for r in range(5):
        sl = slice(r * 8, (r + 1) * 8)
        nc.vector.max(out=s2v[:, sl], in_=cm_val[:])
        nc.vector.max_index(out=s2p[:, sl], in_max=s2v[:, sl], in_values=cm_val[:])
        if r < 4:
            nc.vector.match_replace(out=cm_val[:], in_to_replace=s2v[:, sl],
                                    in_values=cm_val[:], imm_value=SENT)

    s2pf = cp.tile([G, K2], F32)
    nc.vector.tensor_copy(out=s2pf[:], in_=s2p[:])
    g784 = cp.tile([G, 1], I32)
    nc.gpsimd.iota(g784[:], pattern=[[0, 1]], base=0, channel_multiplier=2 * S1)
    g784f = cp.tile([G, 1], F32)
    nc.vector.tensor_copy(out=g784f[:], in_=g784[:])
    nc.vector.tensor_scalar_add(s2pf[:], s2pf[:], g784f[:, 0:1])
    s2pi = cp.tile([G, K2], I32)
    nc.vector.tensor_copy(out=s2pi[:], in_=s2pf[:])

    # HW indirect DMA contract: ONE offset per partition ([P,1] column),
    # each partition moves its free-dim run. All candidate gathers/scatters
    # are therefore column loops. Candidates (g,j) pack to [128, PK] with
    # c = g*K2+j -> partition p = c//PK = 2g + j//PK, slot s = c%PK, so each
    # packed partition belongs to exactly one gt (g = p//2).
    wcs = nc.sync.dma_start(CSD[:], cm_idx[:])
    s2pi_p = cp.tile([128, PK], I32)
    nc.sync.dma_start(s2pi_p[:], s2pi[:])
    gidx_p = cp.tile([128, PK], F32)
    prev = wcs
    for s in range(PK):
        gi_i = nc.gpsimd.indirect_dma_start(
            out=gidx_p[:, s:s + 1], out_offset=None,
            in_=CSD[:].unsqueeze(1),
            in_offset=IndirectOffsetOnAxis(ap=s2pi_p[:, s:s + 1], axis=0))
        add_dep_helper(gi_i.ins, prev.ins, reason="indirect chain")
        prev = gi_i
    gidxi_p = cp.tile([128, PK], I32)
    nc.vector.tensor_copy(out=gidxi_p[:], in_=gidx_p[:])
    crd_p = cp.tile([128, PK * 4], F32)
    for s in range(PK):
        gi_i = nc.gpsimd.indirect_dma_start(
            out=crd_p[:, s * 4:(s + 1) * 4], out_offset=None,
            in_=A[:],
            in_offset=IndirectOffsetOnAxis(ap=gidxi_p[:, s:s + 1], axis=0))
        add_dep_helper(gi_i.ins, prev.ins, reason="indirect chain")
        prev = gi_i
    # unpack to gt-major [64, K2] (same flat order)
    gidxf = cp.tile([G, K2], F32)
    nc.sync.dma_start(gidxf[:], gidx_p[:])
    crd = cp.tile([G, K2 * 4], F32)
    nc.sync.dma_start(crd[:], crd_p[:])
    cv = crd[:].rearrange("p (a f) -> p a f", f=4)

    # ---------------- exact d2 + exact top-27 mask ----------------
    ccx = cp.tile([G, K2], F32)
    ccy = cp.tile([G, K2], F32)
    nc.vector.tensor_tensor(out=ccx[:], in0=cv[:, :, 2], in1=cv[:, :, 0], op=Alu.add)
    nc.vector.tensor_scalar_mul(ccx[:], ccx[:], 0.5)
    nc.vector.tensor_tensor(out=ccy[:], in0=cv[:, :, 3], in1=cv[:, :, 1], op=Alu.add)
    nc.vector.tensor_scalar_mul(ccy[:], ccy[:], 0.5)
    dx = cp.tile([G, K2], F32)
    dy = cp.tile([G, K2], F32)
    nc.vector.tensor_scalar(dx[:], ccx[:], gcx[:, 0:1], None, op0=Alu.subtract)
    nc.vector.tensor_scalar(dy[:], ccy[:], gcy[:, 0:1], None, op0=Alu.subtract)
    work2 = cp.tile([G, K2], F32)
    t2 = cp.tile([G, K2], F32)
    nc.vector.tensor_tensor(out=work2[:], in0=dx[:], in1=dx[:], op=Alu.mult)
    nc.vector.tensor_tensor(out=t2[:], in0=dy[:], in1=dy[:], op=Alu.mult)
    nc.vector.tensor_tensor(out=work2[:], in0=work2[:], in1=t2[:], op=Alu.add)
    nc.vector.tensor_scalar_mul(work2[:], work2[:], -1.0)

    m8t = cp.tile([G, 8], F32)
    for r in range(4):
        nc.vector.max(out=m8t[:], in_=work2[:])
        if r == 3:
            nc.vector.memset(m8t[:, 3:8], DEAD)
        nc.vector.match_replace(out=work2[:], in_to_replace=m8t[:],
                                in_values=work2[:], imm_value=SENT)
    maskf = cp.tile([G, K2], F32)
    nc.vector.tensor_scalar(maskf[:], work2[:], SENT, None, op0=Alu.is_equal)

    # ---------------- iou ----------------
    ltx = cp.tile([G, K2], F32)
    lty = cp.tile([G, K2], F32)
    rbx = cp.tile([G, K2], F32)
    rby = cp.tile([G, K2], F32)
    nc.vector.tensor_scalar(ltx[:], cv[:, :, 0], GT[:, 0:1], None, op0=Alu.max)
    nc.vector.tensor_scalar(lty[:], cv[:, :, 1], GT[:, 1:2], None, op0=Alu.max)
    nc.vector.tensor_scalar(rbx[:], cv[:, :, 2], GT[:, 2:3], None, op0=Alu.min)
    nc.vector.tensor_scalar(rby[:], cv[:, :, 3], GT[:, 3:4], None, op0=Alu.min)
    ww = cp.tile([G, K2], F32)
    hh = cp.tile([G, K2], F32)
    nc.vector.tensor_tensor(out=ww[:], in0=rbx[:], in1=ltx[:], op=Alu.subtract)
    nc.vector.tensor_scalar(ww[:], ww[:], 0.0, None, op0=Alu.max)
    nc.vector.tensor_tensor(out=hh[:], in0=rby[:], in1=lty[:], op=Alu.subtract)
    nc.vector.tensor_scalar(hh[:], hh[:], 0.0, None, op0=Alu.max)
    inter = cp.tile([G, K2], F32)
    nc.vector.tensor_tensor(out=inter[:], in0=ww[:], in1=hh[:], op=Alu.mult)
    aw = cp.tile([G, K2], F32)
    ah = cp.tile([G, K2], F32)
    nc.vector.tensor_tensor(out=aw[:], in0=cv[:, :, 2], in1=cv[:, :, 0], op=Alu.subtract)
    nc.vector.tensor_tensor(out=ah[:], in0=cv[:, :, 3], in1=cv[:, :, 1], op=Alu.subtract)
    aarea = cp.tile([G, K2], F32)
    nc.vector.tensor_tensor(out=aarea[:], in0=aw[:], in1=ah[:], op=Alu.mult)
    union = cp.tile([G, K2], F32)
    nc.vector.tensor_scalar(union[:], aarea[:], gar[:, 0:1], None, op0=Alu.add)
    nc.vector.tensor_tensor(out=union[:], in0=union[:], in1=inter[:], op=Alu.subtract)
    # f32 division via reciprocal + Newton + residual correction (DVE has no
    # divide op; this matches the reference's correctly-rounded divide)
    dv_a = cp.tile([G, K2], F32)
    dv_b = cp.tile([G, K2], F32)

    def fdiv(out_ap, num_ap, den_ap):
        ta, tb = dv_a[:, :num_ap.shape[1]], dv_b[:, :num_ap.shape[1]]
        nc.vector.reciprocal(ta, den_ap)
        nc.vector.tensor_tensor(out=tb, in0=den_ap, in1=ta, op=Alu.mult)
        nc.vector.tensor_scalar(tb, tb, -1.0, 2.0, op0=Alu.mult, op1=Alu.add)
        nc.vector.tensor_tensor(out=ta, in0=ta, in1=tb, op=Alu.mult)
        nc.vector.tensor_tensor(out=out_ap, in0=num_ap, in1=ta, op=Alu.mult)
        nc.vector.tensor_tensor(out=tb, in0=den_ap, in1=out_ap, op=Alu.mult)
        nc.vector.tensor_tensor(out=tb, in0=num_ap, in1=tb, op=Alu.subtract)
        nc.vector.tensor_tensor(out=tb, in0=tb, in1=ta, op=Alu.mult)
        nc.vector.tensor_tensor(out=out_ap, in0=out_ap, in1=tb, op=Alu.add)

    iou = cp.tile([G, K2], F32)
    fdiv(iou[:], inter[:], union[:])

    # ---------------- center-inside-gt ----------------
    li = cp.tile([G, K2], F32)
    ti = cp.tile([G, K2], F32)
    ri = cp.tile([G, K2], F32)
    bi = cp.tile([G, K2], F32)
    nc.vector.tensor_scalar(li[:], ccx[:], GT[:, 0:1], None, op0=Alu.subtract)
    nc.vector.tensor_scalar(ti[:], ccy[:], GT[:, 1:2], None, op0=Alu.subtract)
    nc.vector.scalar_tensor_tensor(out=ri[:], in0=ccx[:], scalar=-1.0,
                                   in1=GT[:, 2:3].to_broadcast([G, K2]),
                                   op0=Alu.mult, op1=Alu.add)
    nc.vector.scalar_tensor_tensor(out=bi[:], in0=ccy[:], scalar=-1.0,
                                   in1=GT[:, 3:4].to_broadcast([G, K2]),
                                   op0=Alu.mult, op1=Alu.add)
    nc.vector.tensor_tensor(out=li[:], in0=li[:], in1=ti[:], op=Alu.min)
    nc.vector.tensor_tensor(out=ri[:], in0=ri[:], in1=bi[:], op=Alu.min)
    nc.vector.tensor_tensor(out=li[:], in0=li[:], in1=ri[:], op=Alu.min)
    insf = cp.tile([G, K2], F32)
    nc.vector.tensor_scalar(insf[:], li[:], 0.01, None, op0=Alu.is_gt)

    # ---------------- mean/std over top-27 ----------------
    mi = cp.tile([G, K2], F32)
    nc.vector.tensor_tensor(out=mi[:], in0=iou[:], in1=maskf[:], op=Alu.mult)
    msum = cp.tile([G, 1], F32)
    nc.vector.tensor_reduce(out=msum[:], in_=mi[:], axis=mybir.AxisListType.X, op=Alu.add)
    mean = cp.tile([G, 1], F32)
    c27 = cp.tile([G, 1], F32)
    nc.vector.memset(c27[:], float(TOPK))
    fdiv(mean[:], msum[:], c27[:])
    dev = cp.tile([G, K2], F32)
    nc.vector.tensor_scalar(dev[:], iou[:], mean[:, 0:1], None, op0=Alu.subtract)
    nc.vector.tensor_tensor(out=dev[:], in0=dev[:], in1=maskf[:], op=Alu.mult)
    nc.vector.tensor_tensor(out=dev[:], in0=dev[:], in1=dev[:], op=Alu.mult)
    vsum = cp.tile([G, 1], F32)
    nc.vector.tensor_reduce(out=vsum[:], in_=dev[:], axis=mybir.AxisListType.X, op=Alu.add)
    c26 = cp.tile([G, 1], F32)
    nc.vector.memset(c26[:], float(TOPK - 1))
    vvar = cp.tile([G, 1], F32)
    fdiv(vvar[:], vsum[:], c26[:])
    stdt = cp.tile([G, 1], F32)
    nc.scalar.activation(stdt[:], vvar[:], Act.Sqrt)
    thr = cp.tile([G, 1], F32)
    nc.vector.tensor_tensor(out=thr[:], in0=mean[:], in1=stdt[:], op=Alu.add)

    # ---------------- is_pos ----------------
    ispos = cp.tile([G, K2], F32)
    nc.vector.tensor_scalar(ispos[:], iou[:], thr[:, 0:1], None, op0=Alu.is_ge)
    nc.vector.tensor_tensor(out=ispos[:], in0=ispos[:], in1=maskf[:], op=Alu.mult)
    nc.vector.tensor_tensor(out=ispos[:], in0=ispos[:], in1=insf[:], op=Alu.mult)

    # ---------------- scatter-max values ----------------
    # ---------------- pack to [128, PK] for the fixup/scatter phase -------
    iou_p = cp.tile([128, PK], F32)
    nc.sync.dma_start(iou_p[:], iou[:])
    isp_p = cp.tile([128, PK], F32)
    nc.sync.dma_start(isp_p[:], ispos[:])
    gmb_p = cp.tile([128, PK], F32)
    nc.vector.tensor_scalar(gmb_p[:], gidx_p[:], BIG_OFF, None, op0=Alu.subtract)
    offp = cp.tile([128, PK], F32)
    nc.vector.tensor_tensor(out=offp[:], in0=gmb_p[:], in1=isp_p[:], op=Alu.mult)
    nc.vector.tensor_scalar(offp[:], offp[:], BIG_OFF, None, op0=Alu.add)
    offpi = cp.tile([128, PK], I32)
    nc.vector.tensor_copy(out=offpi[:], in_=offp[:])

    # Iterative scatter-with-fixup into the values output: pass 1 scatters
    # every positive; pass 2 rescatters candidates strictly above the buffer
    # value. Buffer strictly increases at contested anchors each pass (any
    # write order), reaching the per-anchor max in <= multiplicity passes
    # (multiplicity is 2 at most: gt centers must nearly coincide).
    for s in range(PK):
        si = nc.gpsimd.indirect_dma_start(
            out=OV[:].unsqueeze(1),
            out_offset=IndirectOffsetOnAxis(ap=offpi[:, s:s + 1], axis=0),
            in_=iou_p[:, s:s + 1], in_offset=None,
            bounds_check=N - 1, oob_is_err=False)
        add_dep_helper(si.ins, prev.ins, reason="indirect chain")
        add_dep_helper(si.ins, fill_insts['v'][0].ins, reason="after vals fill")
        prev = si
    gat_p = cp.tile([128, PK], F32)
    mgt = cp.tile([128, PK], F32)
    for p_ in range(NPASS - 1):
        for s in range(PK):
            gi2 = nc.gpsimd.indirect_dma_start(
                out=gat_p[:, s:s + 1], out_offset=None,
                in_=OV[:].unsqueeze(1),
                in_offset=IndirectOffsetOnAxis(ap=offpi[:, s:s + 1], axis=0),
                bounds_check=N - 1, oob_is_err=False)
            add_dep_helper(gi2.ins, prev.ins, reason="indirect chain")
            prev = gi2
        nc.vector.tensor_tensor(out=mgt[:], in0=iou_p[:], in1=gat_p[:], op=Alu.is_gt)
        nc.vector.tensor_tensor(out=mgt[:], in0=mgt[:], in1=isp_p[:], op=Alu.mult)
        poff = cp.tile([128, PK], F32, tag="poff")
        nc.vector.tensor_tensor(out=poff[:], in0=gmb_p[:], in1=mgt[:], op=Alu.mult)
        nc.vector.tensor_scalar(poff[:], poff[:], BIG_OFF, None, op0=Alu.add)
        poffi = cp.tile([128, PK], I32, tag="poffi")
        nc.vector.tensor_copy(out=poffi[:], in_=poff[:])
        for s in range(PK):
            si = nc.gpsimd.indirect_dma_start(
                out=OV[:].unsqueeze(1),
                out_offset=IndirectOffsetOnAxis(ap=poffi[:, s:s + 1], axis=0),
                in_=iou_p[:, s:s + 1], in_offset=None,
                bounds_check=N - 1, oob_is_err=False)
            add_dep_helper(si.ins, prev.ins, reason="indirect chain")
            prev = si
    for s in range(PK):
        gi3 = nc.gpsimd.indirect_dma_start(
            out=gat_p[:, s:s + 1], out_offset=None,
            in_=OV[:].unsqueeze(1),
            in_offset=IndirectOffsetOnAxis(ap=offpi[:, s:s + 1], axis=0),
            bounds_check=N - 1, oob_is_err=False)
        add_dep_helper(gi3.ins, prev.ins, reason="indirect chain")
        prev = gi3

    # ---------------- winners + final scatters ----------------
    win_p = cp.tile([128, PK], F32)
    nc.vector.tensor_tensor(out=win_p[:], in0=gat_p[:], in1=iou_p[:], op=Alu.is_equal)
    nc.vector.tensor_tensor(out=win_p[:], in0=win_p[:], in1=isp_p[:], op=Alu.mult)
    woff = cp.tile([128, PK], F32)
    nc.vector.tensor_tensor(out=woff[:], in0=gmb_p[:], in1=win_p[:], op=Alu.mult)
    nc.vector.tensor_scalar(woff[:], woff[:], BIG_OFF, None, op0=Alu.add)
    woffi = cp.tile([128, PK], I32)
    nc.vector.tensor_copy(out=woffi[:], in_=woff[:])

    # packed partition p holds candidates of gt p//2
    gcol = cp.tile([128, 1], I32)
    nc.gpsimd.iota(gcol[:], pattern=[[0, 1]], base=0, channel_multiplier=1)
    nc.vector.tensor_scalar(gcol[:], gcol[:], 1, None, op0=Alu.arith_shift_right)
    gval_p = cp.tile([128, PK], I32)
    nc.vector.tensor_copy(out=gval_p[:], in_=gcol[:].to_broadcast([128, PK]))
    grow = cp.tile([128, 4], F32)
    gi4 = nc.gpsimd.indirect_dma_start(
        out=grow[:], out_offset=None, in_=GTS[:],
        in_offset=IndirectOffsetOnAxis(ap=gcol[:], axis=0))
    add_dep_helper(gi4.ins, prev.ins, reason="indirect chain")
    prev = gi4
    trow = cp.tile([128, T], F32)
    gi5 = nc.gpsimd.indirect_dma_start(
        out=trow[:], out_offset=None, in_=TOK[:],
        in_offset=IndirectOffsetOnAxis(ap=gcol[:], axis=0))
    add_dep_helper(gi5.ins, prev.ins, reason="indirect chain")
    prev = gi5

    for s in range(PK):
        si = nc.gpsimd.indirect_dma_start(
            out=OI[:].unsqueeze(1),
            out_offset=IndirectOffsetOnAxis(ap=woffi[:, s:s + 1], axis=0),
            in_=gval_p[:, s:s + 1], in_offset=None,
            bounds_check=N - 1, oob_is_err=False)
        add_dep_helper(si.ins, prev.ins, reason="indirect chain")
        add_dep_helper(si.ins, fill_insts['i'][0].ins, reason="after idx fill")
        prev = si
    for s in range(PK):
        si = nc.gpsimd.indirect_dma_start(
            out=OM[:],
            out_offset=IndirectOffsetOnAxis(ap=woffi[:, s:s + 1], axis=0),
            in_=grow[:], in_offset=None,
            bounds_check=N - 1, oob_is_err=False)
        add_dep_helper(si.ins, prev.ins, reason="indirect chain")
        for f in fill_insts['m']:
            add_dep_helper(si.ins, f.ins, reason="after mgts fill")
        prev = si
    for s in range(PK):
        si = nc.gpsimd.indirect_dma_start(
            out=OT[:],
            out_offset=IndirectOffsetOnAxis(ap=woffi[:, s:s + 1], axis=0),
            in_=trow[:], in_offset=None,
            bounds_check=N - 1, oob_is_err=False)
        add_dep_helper(si.ins, prev.ins, reason="indirect chain")
        for f in fill_insts['t']:
            add_dep_helper(si.ins, f.ins, reason="after tokens fill")
        prev = si


def build(debug=False):
    nc = bacc.Bacc("TRN2", target_bir_lowering=False, debug=debug)
    A = nc.dram_tensor("anchors", [N, 4], F32, kind="ExternalInput")
    GTS = nc.dram_tensor("gts", [G, 4], F32, kind="ExternalInput")
    TOK = nc.dram_tensor("tokens", [G, T], F32, kind="ExternalInput")
    OV = nc.dram_tensor("out_vals", [N], F32, kind="ExternalOutput")
    OI = nc.dram_tensor("out_idxs", [N], I32, kind="ExternalOutput")
    OM = nc.dram_tensor("out_mgts", [N, 4], F32, kind="ExternalOutput")
    OT = nc.dram_tensor("out_tok", [N, T], F32, kind="ExternalOutput")
    FD = nc.dram_tensor("featd", [4, NP], F32)
    LTD = nc.dram_tensor("lhsTd", [4, G], F32)
    CSD = nc.dram_tensor("candidxd", [G * 2 * CPAIR * 8], F32)
    from contextlib import ExitStack
    with tile.TileContext(nc) as tc, ExitStack() as ctx:
        _emit(tc, ctx, A[:], GTS[:], TOK[:], OV[:], OI[:], OM[:], OT[:],
              FD, LTD, CSD)
    nc.compile()
    return nc


_nc = None
LAST_EXEC_NS = None
LAST_PROFILE = None


def kernel(anchor_boxes, gt_boxes, token_map):
    global _nc, LAST_EXEC_NS, LAST_PROFILE
    if _nc is None:
        _nc = build(debug=False)
    in_maps = [{
        "anchors": np.ascontiguousarray(anchor_boxes[b], dtype=np.float32),
        "gts": np.ascontiguousarray(gt_boxes[b], dtype=np.float32),
        "tokens": np.ascontiguousarray(token_map[b], dtype=np.float32),
    } for b in range(B)]
    import os
    kw = {}
    if os.environ.get("KERNEL_TRACE"):
        kw = dict(trace=True, tmpdir=os.environ.get("KERNEL_TRACE_DIR") or None)
    r = bass_utils.run_bass_kernel_spmd(_nc, in_maps, list(range(B)), **kw)
    LAST_EXEC_NS = r.exec_time_ns
    LAST_PROFILE = r.profile_json
    res = r.results
    vals = np.stack([res[b]["out_vals"] for b in range(B)])
    idxs = np.stack([res[b]["out_idxs"] for b in range(B)])
    mgts = np.stack([res[b]["out_mgts"] for b in range(B)])
    toks = np.stack([res[b]["out_tok"] for b in range(B)])
    return vals, idxs.astype(np.int32), mgts, toks
